# revision 3
# baseline (speedup 1.0000x reference)
"""DeepseekV2 MLA prefill kernel for 8 Trainium2 NeuronCores.

Sharding: token-parallel over causal query blocks. Core c owns query blocks
{c, 15-c} (128 tokens each -> balanced causal work: 17 key-blocks per core).
Each core computes the shared latent KV only for its own 256 tokens; the tiny
[576, 256] latent-KV slice is AllGathered (fp32r, ~590KB/rank), then every
core runs all 32 heads over its own query blocks and the full o_proj for its
tokens. The host reassembles disjoint token slices - no all-reduce needed.

SPMD note: all cores run one program, so per-core causal lengths are handled
by padding scores to fixed widths (1024 for the low block, 2048 for the high
block) and zeroing the surplus via per-core uploaded column masks.

All matmuls run in fp32r (fp32 with 11-bit mantissa, full PE rate at N>=256).
Weights are pre-rounded host-side; rmsnorm ln-weights are folded into the
adjacent weight matrices; RoPE pair de-interleave is folded into weight column
permutations; cos/sin tables are computed from `positions` on the host.
"""

import os
import numpy as np

import concourse.bacc as bacc
import concourse.bass as bass
import concourse.mybir as mybir
import concourse.tile as tile
from concourse.bass_utils import run_bass_kernel_spmd
from concourse.masks import make_identity

F32R = mybir.dt.float32r
F32 = mybir.dt.float32
AOP = mybir.AluOpType
AFT = mybir.ActivationFunctionType

HID = 5120
QLR = 1536
KVLR = 512
NOPE = 128
ROPE = 64
VDIM = 128
H = 32
T = 2048
QKD = NOPE + ROPE
SCALE = QKD ** -0.5
EPS = 1e-6
THETA = 10000.0

N_CORES = 8
NB = T // 128          # 16 key/query blocks of 128
KD = HID // 128        # 40
KQ = QLR // 128        # 12
LKV = KVLR + ROPE      # 576
KMAX = (1024, 2048)    # padded score widths for block slots {c, 15-c}

_last_results = None


def round_fp32r(x):
    b = np.ascontiguousarray(x, dtype=np.float32).view(np.uint32)
    lsb = (b >> 12) & 1
    return ((b + np.uint32(0x7FF) + lsb) & np.uint32(0xFFFFF000)).view(np.float32)


def _pe_perm():
    return np.concatenate([np.arange(0, ROPE, 2), np.arange(1, ROPE, 2)])


def _build():
    nc = bacc.Bacc("TRN2", target_bir_lowering=False, debug=False,
                   num_devices=N_CORES)

    hid = nc.dram_tensor("hid", [256, HID], F32R, kind="ExternalInput").ap()
    wqa = nc.dram_tensor("wqa", [HID, QLR], F32R, kind="ExternalInput").ap()
    wqb = nc.dram_tensor("wqb", [QLR, H * QKD], F32R, kind="ExternalInput").ap()
    wkva = nc.dram_tensor("wkva", [HID, LKV], F32R, kind="ExternalInput").ap()
    wkc = nc.dram_tensor("wkc", [H, NOPE, KVLR], F32R, kind="ExternalInput").ap()
    wvc = nc.dram_tensor("wvc", [H, KVLR, VDIM], F32R, kind="ExternalInput").ap()
    wo = nc.dram_tensor("wo", [H * VDIM, HID], F32R, kind="ExternalInput").ap()
    cs_own = nc.dram_tensor("cs_own", [256, ROPE], F32R, kind="ExternalInput").ap()
    csrep = nc.dram_tensor("csrep", [256, 2 * H * 32], F32R,
                           kind="ExternalInput").ap()
    bmask = nc.dram_tensor("bmask", [128, KMAX[0] + KMAX[1]], F32R,
                           kind="ExternalInput").ap()
    out = nc.dram_tensor("outT", [HID, 256], F32, kind="ExternalOutput").ap()

    with tile.TileContext(nc) as tc:
        with tc.tile_pool(name="outer", bufs=1) as pp, \
             tc.tile_pool(name="dram", bufs=1, space="DRAM") as dram:
            # ---- constants + tensors alive through P6 ----
            ident_f = pp.tile([128, 128], F32)
            make_identity(nc, ident_f[:])
            ident = pp.tile([128, 128], F32R)
            nc.vector.tensor_copy(ident[:], ident_f[:])
            eps_t = pp.tile([128, 1], F32)
            nc.gpsimd.memset(eps_t[:], EPS)
            cs_sb = pp.tile([128, 2, ROPE], F32R)
            nc.sync.dma_start(cs_sb[:], cs_own.rearrange("(b p) r -> p b r", p=128))
            attn_all = pp.tile([128, H, 256], F32R)

            in_blob = dram.tile([LKV, 256], F32R)
            gblob = dram.tile([N_CORES * LKV, 256], F32R, addr_space="Shared")

            with tc.tile_pool(name="kvpool", bufs=1) as kv:
                kT = [kv.tile([128, T], F32R, name=f"kT{i}") for i in range(4)]
                kTpe = kv.tile([64, T], F32R)
                qanT = kv.tile([128, KQ, 256], F32R)

                # ======== P1 + P2 (hT-scoped) ========
                with tc.tile_pool(name="hpool", bufs=1) as hp:
                    hT = hp.tile([128, 2, KD, 128], F32R)
                    kTo = [hp.tile([128, 256], F32R, name=f"kTo{i}")
                           for i in range(4)]
                    kToPe = hp.tile([64, 256], F32R)
                    with tc.tile_pool(name="p1ps", bufs=1, space="PSUM") as p1ps:
                        for b in range(2):
                            # transpose own hidden rows -> hT (stream quarters)
                            for q4 in range(4):
                                hsb = hp.tile([128, HID // 4], F32R,
                                              tag="hsb", bufs=2)
                                nc.sync.dma_start(
                                    hsb[:], hid[b * 128:(b + 1) * 128,
                                                q4 * (HID // 4):(q4 + 1) * (HID // 4)])
                                for kk in range(KD // 4):
                                    k = q4 * (KD // 4) + kk
                                    trp = p1ps.tile([128, 128], F32R,
                                                    tag="tr", bufs=2)
                                    nc.tensor.transpose(
                                        trp[:], hsb[:, kk * 128:(kk + 1) * 128],
                                        ident[:])
                                    nc.vector.tensor_copy(hT[:, b, k, :], trp[:])
                            # latent = hid @ wkva (token-major)
                            psA = p1ps.tile([128, 512], F32, tag="acc", bufs=2)
                            psB = p1ps.tile([128, 64], F32, tag="accB", bufs=2)
                            for k in range(KD):
                                wk = hp.tile([128, LKV], F32R, tag="wkva", bufs=4)
                                nc.sync.dma_start(
                                    wk[:], wkva[k * 128:(k + 1) * 128, :])
                                nc.tensor.matmul(psA[:], hT[:, b, k, :],
                                                 wk[:, 0:KVLR], start=(k == 0),
                                                 stop=(k == KD - 1),
                                                 skip_group_check=True)
                                nc.tensor.matmul(psB[:], hT[:, b, k, :],
                                                 wk[:, KVLR:LKV], start=(k == 0),
                                                 stop=(k == KD - 1),
                                                 skip_group_check=True)
                            # rmsnorm(ckv)
                            ckv = hp.tile([128, KVLR], F32, tag="ckv", bufs=2)
                            nc.vector.tensor_copy(ckv[:], psA[:])
                            sq = hp.tile([128, QLR], F32, tag="sqscr", bufs=1)
                            ssq = hp.tile([128, 1], F32, tag="ssq", bufs=2)
                            nc.scalar.activation(sq[:, 0:KVLR], ckv[:], AFT.Square,
                                                 accum_out=ssq[:])
                            den = hp.tile([128, 1], F32, tag="den", bufs=2)
                            nc.scalar.activation(den[:], ssq[:], AFT.Sqrt,
                                                 scale=1.0 / KVLR, bias=eps_t[:])
                            fac = hp.tile([128, 1], F32, tag="fac", bufs=2)
                            nc.vector.reciprocal(fac[:], den[:])
                            vown = hp.tile([128, KVLR], F32R, tag="vown", bufs=2)
                            nc.vector.tensor_scalar_mul(vown[:], ckv[:], fac[:])
                            # rope(k_pe) - psB holds deinterleaved [x1|x2]
                            kpe = hp.tile([128, ROPE], F32R, tag="kpe", bufs=2)
                            t1 = hp.tile([128, 32], F32, tag="t1", bufs=2)
                            t2 = hp.tile([128, 32], F32, tag="t2", bufs=2)
                            cosv = cs_sb[:, b, 0:32]
                            sinv = cs_sb[:, b, 32:64]
                            nc.vector.tensor_tensor(out=t1[:], in0=psB[:, 0:32],
                                                    in1=cosv, op=AOP.mult)
                            nc.vector.tensor_tensor(out=t2[:], in0=psB[:, 32:64],
                                                    in1=sinv, op=AOP.mult)
                            nc.vector.tensor_tensor(out=kpe[:, 0:32], in0=t1[:],
                                                    in1=t2[:], op=AOP.subtract)
                            nc.vector.tensor_tensor(out=t1[:], in0=psB[:, 32:64],
                                                    in1=cosv, op=AOP.mult)
                            nc.vector.tensor_tensor(out=t2[:], in0=psB[:, 0:32],
                                                    in1=sinv, op=AOP.mult)
                            nc.vector.tensor_tensor(out=kpe[:, 32:64], in0=t1[:],
                                                    in1=t2[:], op=AOP.add)
                            # transpose vown + kpe -> own latent-KV columns
                            for kc in range(4):
                                trp = p1ps.tile([128, 128], F32R, tag="tr", bufs=2)
                                nc.tensor.transpose(
                                    trp[:], vown[:, kc * 128:(kc + 1) * 128],
                                    ident[:])
                                nc.vector.tensor_copy(
                                    kTo[kc][:, b * 128:(b + 1) * 128], trp[:])
                            trp2 = p1ps.tile([64, 128], F32R, tag="tr", bufs=2)
                            nc.tensor.transpose(trp2[:], kpe[:], ident[:])
                            nc.vector.tensor_copy(
                                kToPe[:, b * 128:(b + 1) * 128], trp2[:])

                        # gather latent KV across cores
                        for kc in range(4):
                            nc.gpsimd.dma_start(
                                in_blob[kc * 128:(kc + 1) * 128, :], kTo[kc][:])
                        nc.gpsimd.dma_start(in_blob[KVLR:LKV, :], kToPe[:])
                        nc.gpsimd.collective_compute(
                            "AllGather", AOP.bypass,
                            replica_groups=[list(range(N_CORES))],
                            ins=[in_blob.opt()], outs=[gblob.opt()],
                        )
                        for r in range(N_CORES):
                            for pos, gb in ((0, r), (1, 15 - r)):
                                src = gblob[r * LKV:(r + 1) * LKV,
                                            pos * 128:(pos + 1) * 128]
                                for kc in range(4):
                                    nc.sync.dma_start(
                                        kT[kc][:, gb * 128:(gb + 1) * 128],
                                        src[kc * 128:(kc + 1) * 128, :])
                                nc.sync.dma_start(
                                    kTpe[:, gb * 128:(gb + 1) * 128],
                                    src[KVLR:LKV, :])

                    # ---- P2: q_a for own tokens (uses hT) ----
                    with tc.tile_pool(name="p2ps", bufs=1, space="PSUM") as p2ps:
                        qaps = [[p2ps.tile([128, 512], F32, tag=f"qa{b}{j}",
                                           name=f"qaps{b}{j}")
                                 for j in range(3)] for b in range(2)]
                        for k in range(KD):
                            wq = hp.tile([128, QLR], F32R, tag="wqa", bufs=3)
                            nc.sync.dma_start(wq[:], wqa[k * 128:(k + 1) * 128, :])
                            for b in range(2):
                                for j in range(3):
                                    nc.tensor.matmul(
                                        qaps[b][j][:], hT[:, b, k, :],
                                        wq[:, j * 512:(j + 1) * 512],
                                        start=(k == 0), stop=(k == KD - 1),
                                        skip_group_check=True)
                        for b in range(2):
                            qa_sb = hp.tile([128, QLR], F32, tag="qasb", bufs=1)
                            for j in range(3):
                                nc.vector.tensor_copy(
                                    qa_sb[:, j * 512:(j + 1) * 512], qaps[b][j][:])
                            sq2 = hp.tile([128, QLR], F32, tag="sqscr", bufs=1)
                            ss2 = hp.tile([128, 1], F32, tag="ss2", bufs=2)
                            nc.scalar.activation(sq2[:], qa_sb[:], AFT.Square,
                                                 accum_out=ss2[:])
                            den2 = hp.tile([128, 1], F32, tag="den2", bufs=2)
                            nc.scalar.activation(den2[:], ss2[:], AFT.Sqrt,
                                                 scale=1.0 / QLR, bias=eps_t[:])
                            fac2 = hp.tile([128, 1], F32, tag="fac2", bufs=2)
                            nc.vector.reciprocal(fac2[:], den2[:])
                            qan = hp.tile([128, QLR], F32R, tag="qan", bufs=1)
                            nc.vector.tensor_scalar_mul(qan[:], qa_sb[:], fac2[:])
                            for j12 in range(KQ):
                                trp = p2ps.tile([128, 128], F32R, tag="tr", bufs=2)
                                nc.tensor.transpose(
                                    trp[:], qan[:, j12 * 128:(j12 + 1) * 128],
                                    ident[:])
                                nc.vector.tensor_copy(
                                    qanT[:, j12, b * 128:(b + 1) * 128], trp[:])

                # ======== P3: q = q_a_n @ wqb (token-major), hpool freed ======
                with tc.tile_pool(name="vpool", bufs=1) as vp:
                    q_all = vp.tile([128, 2, H * QKD], F32R)
                    vall = vp.tile([128, NB, KVLR], F32R)
                    with tc.tile_pool(name="p3", bufs=1) as p3, \
                         tc.tile_pool(name="p3ps", bufs=1, space="PSUM") as p3ps:
                        for n in range(KQ):
                            ps0 = p3ps.tile([128, 512], F32, tag="qb0", bufs=2)
                            ps1 = p3ps.tile([128, 512], F32, tag="qb1", bufs=2)
                            for k in range(KQ):
                                wb = p3.tile([128, 512], F32R, tag="wqb", bufs=6)
                                nc.sync.dma_start(
                                    wb[:], wqb[k * 128:(k + 1) * 128,
                                               n * 512:(n + 1) * 512])
                                nc.tensor.matmul(ps0[:], qanT[:, k, 0:128], wb[:],
                                                 start=(k == 0),
                                                 stop=(k == KQ - 1),
                                                 skip_group_check=True)
                                nc.tensor.matmul(ps1[:], qanT[:, k, 128:256],
                                                 wb[:], start=(k == 0),
                                                 stop=(k == KQ - 1),
                                                 skip_group_check=True)
                            nc.vector.tensor_copy(
                                q_all[:, 0, n * 512:(n + 1) * 512], ps0[:])
                            nc.vector.tensor_copy(
                                q_all[:, 1, n * 512:(n + 1) * 512], ps1[:])

                    # ---- P2b: derive token-major v from gathered kT ----
                    with tc.tile_pool(name="p2bps", bufs=1, space="PSUM") as p2b:
                        for sb in range(NB):
                            for kc in range(4):
                                trp = p2b.tile([128, 128], F32R, tag="tr", bufs=4)
                                nc.tensor.transpose(
                                    trp[:], kT[kc][:, sb * 128:(sb + 1) * 128],
                                    ident[:])
                                nc.vector.tensor_copy(
                                    vall[:, sb, kc * 128:(kc + 1) * 128], trp[:])

                    # ---- P4: rope on q_pe across all heads ----
                    with tc.tile_pool(name="p4", bufs=1) as p4:
                        crep = p4.tile([128, 2, H * 32], F32R, tag="crep")
                        srep = p4.tile([128, 2, H * 32], F32R, tag="srep")
                        csr = csrep.rearrange("(b p) (c x) -> p b c x", p=128, c=2)
                        nc.sync.dma_start(crep[:], csr[:, :, 0, :])
                        nc.sync.dma_start(srep[:], csr[:, :, 1, :])
                        for b in range(2):
                            base = q_all[:, b]
                            x1 = bass.AP(base.tensor, base.offset + NOPE,
                                         [base.ap[0], [QKD, H], [1, 32]])
                            x2 = bass.AP(base.tensor, base.offset + NOPE + 32,
                                         [base.ap[0], [QKD, H], [1, 32]])
                            cosv = crep[:, b].rearrange("p (h x) -> p h x", h=H)
                            sinv = srep[:, b].rearrange("p (h x) -> p h x", h=H)
                            t1 = p4.tile([128, H, 32], F32, tag="t1")
                            t2 = p4.tile([128, H, 32], F32, tag="t2")
                            t3 = p4.tile([128, H, 32], F32, tag="t3")
                            t4 = p4.tile([128, H, 32], F32, tag="t4")
                            nc.vector.tensor_tensor(out=t1[:], in0=x1, in1=cosv,
                                                    op=AOP.mult)
                            nc.vector.tensor_tensor(out=t2[:], in0=x2, in1=sinv,
                                                    op=AOP.mult)
                            nc.vector.tensor_tensor(out=t3[:], in0=x2, in1=cosv,
                                                    op=AOP.mult)
                            nc.vector.tensor_tensor(out=t4[:], in0=x1, in1=sinv,
                                                    op=AOP.mult)
                            nc.vector.tensor_tensor(out=x1, in0=t1[:], in1=t2[:],
                                                    op=AOP.subtract)
                            nc.vector.tensor_tensor(out=x2, in0=t3[:], in1=t4[:],
                                                    op=AOP.add)

                    # ---- P5: attention, head-by-head ----
                    with tc.tile_pool(name="p5", bufs=1) as p5, \
                         tc.tile_pool(name="p5ps", bufs=1, space="PSUM") as p5ps:
                        bm = p5.tile([128, KMAX[0] + KMAX[1]], F32R, tag="bm")
                        nc.sync.dma_start(bm[:], bmask[:])
                        bmx = (bm[:, 0:KMAX[0]], bm[:, KMAX[0]:])
                        for h in range(H):
                            wkc_s = p5.tile([128, KVLR], F32R, tag="wkc", bufs=2)
                            nc.sync.dma_start(wkc_s[:], wkc[h])
                            wvc_s = p5.tile([128, 4, VDIM], F32R, tag="wvc", bufs=2)
                            nc.sync.dma_start(
                                wvc_s[:], wvc[h].rearrange("(c s) v -> s c v",
                                                           s=128))
                            qnT = p5.tile([128, 256], F32R, tag="qnT", bufs=1)
                            qpeT = p5.tile([64, 256], F32R, tag="qpeT", bufs=1)
                            for b in range(2):
                                trp = p5ps.tile([128, 128], F32R, tag="tr", bufs=2)
                                nc.tensor.transpose(
                                    trp[:], q_all[:, b, h * QKD:h * QKD + NOPE],
                                    ident[:])
                                nc.vector.tensor_copy(
                                    qnT[:, b * 128:(b + 1) * 128], trp[:])
                                trp2 = p5ps.tile([64, 128], F32R, tag="tr", bufs=2)
                                nc.tensor.transpose(
                                    trp2[:],
                                    q_all[:, b, h * QKD + NOPE:(h + 1) * QKD],
                                    ident[:])
                                nc.vector.tensor_copy(
                                    qpeT[:, b * 128:(b + 1) * 128], trp2[:])
                            qloT = p5.tile([128, 4, 256], F32R, tag="qloT", bufs=1)
                            for m in range(4):
                                aps = p5ps.tile([128, 256], F32, tag="acc", bufs=2)
                                nc.tensor.matmul(aps[:],
                                                 wkc_s[:, m * 128:(m + 1) * 128],
                                                 qnT[:], start=True, stop=True)
                                nc.vector.tensor_copy(qloT[:, m], aps[:])

                            ctxT = p5.tile([128, 4, 256], F32R, tag="ctxT", bufs=1)
                            for b in range(2):
                                kmax = KMAX[b]
                                nst = kmax // 512
                                sc_ps = p5ps.tile([128, 2048], F32, tag="sc",
                                                  bufs=1)
                                for st in range(nst):
                                    for kc in range(5):
                                        if kc < 4:
                                            lhs = qloT[:, kc][:, b * 128:(b + 1) * 128]
                                            rhs = kT[kc][:, st * 512:(st + 1) * 512]
                                        else:
                                            lhs = qpeT[:, b * 128:(b + 1) * 128]
                                            rhs = kTpe[:, st * 512:(st + 1) * 512]
                                        nc.tensor.matmul(
                                            sc_ps[:, st * 512:(st + 1) * 512],
                                            lhs, rhs, start=(kc == 0),
                                            stop=(kc == 4))
                                probs = p5.tile([128, 2048], F32R, tag="probs",
                                                bufs=1)
                                partials = p5.tile([128, 4], F32, tag="part",
                                                   bufs=2)
                                for st in range(nst):
                                    nc.scalar.activation(
                                        probs[:, st * 512:(st + 1) * 512],
                                        sc_ps[:, st * 512:(st + 1) * 512],
                                        AFT.Exp, scale=SCALE)
                                nc.vector.tensor_tensor(
                                    out=probs[:, 0:kmax], in0=probs[:, 0:kmax],
                                    in1=bmx[b], op=AOP.mult)
                                for st in range(nst):
                                    nc.vector.tensor_reduce(
                                        out=partials[:, st:st + 1],
                                        in_=probs[:, st * 512:(st + 1) * 512],
                                        axis=mybir.AxisListType.X, op=AOP.add)
                                sumexp = p5.tile([128, 1], F32, tag="sume", bufs=2)
                                nc.vector.tensor_reduce(
                                    out=sumexp[:], in_=partials[:, 0:nst],
                                    axis=mybir.AxisListType.X, op=AOP.add)
                                recip = p5.tile([128, 1], F32, tag="rec", bufs=2)
                                nc.vector.reciprocal(recip[:], sumexp[:])
                                ctx_ps = p5ps.tile([128, 512], F32, tag="acc",
                                                   bufs=2)
                                for sc in range(kmax // 128):
                                    trp = p5ps.tile([128, 128], F32R, tag="tr",
                                                    bufs=2)
                                    nc.tensor.transpose(
                                        trp[:], probs[:, sc * 128:(sc + 1) * 128],
                                        ident[:])
                                    pT = p5.tile([128, 128], F32R, tag="pT",
                                                 bufs=2)
                                    nc.vector.tensor_copy(pT[:], trp[:])
                                    nc.tensor.matmul(
                                        ctx_ps[:], pT[:], vall[:, sc],
                                        start=(sc == 0),
                                        stop=(sc == kmax // 128 - 1))
                                ctx_sb = p5.tile([128, KVLR], F32R, tag="ctxsb",
                                                 bufs=1)
                                nc.vector.tensor_scalar_mul(ctx_sb[:], ctx_ps[:],
                                                            recip[:])
                                for rc in range(4):
                                    trp = p5ps.tile([128, 128], F32R, tag="tr",
                                                    bufs=2)
                                    nc.tensor.transpose(
                                        trp[:], ctx_sb[:, rc * 128:(rc + 1) * 128],
                                        ident[:])
                                    nc.vector.tensor_copy(
                                        ctxT[:, rc, b * 128:(b + 1) * 128], trp[:])
                            atps = p5ps.tile([128, 256], F32, tag="acc", bufs=2)
                            for rc in range(4):
                                nc.tensor.matmul(atps[:], wvc_s[:, rc],
                                                 ctxT[:, rc], start=(rc == 0),
                                                 stop=(rc == 3))
                            nc.vector.tensor_copy(attn_all[:, h], atps[:])

            # ======== P6: o_proj (kvpool freed -> room for wo stream) ========
            with tc.tile_pool(name="p6", bufs=1) as p6, \
                 tc.tile_pool(name="p6ps", bufs=1, space="PSUM") as p6ps:
                for m in range(KD):
                    wom = p6.tile([128, H, 128], F32R, tag="wo", bufs=3)
                    nc.sync.dma_start(
                        wom[:], wo[:, m * 128:(m + 1) * 128].rearrange(
                            "(h s) v -> s h v", s=128))
                    ops_ = p6ps.tile([128, 256], F32, tag="acc", bufs=2)
                    for h in range(H):
                        nc.tensor.matmul(ops_[:], wom[:, h], attn_all[:, h],
                                         start=(h == 0), stop=(h == H - 1))
                    osb = p6.tile([128, 256], F32, tag="osb", bufs=3)
                    nc.vector.tensor_copy(osb[:], ops_[:])
                    nc.sync.dma_start(out[m * 128:(m + 1) * 128, :], osb[:])

    nc.compile()
    return nc


_nc_cache = None


def _get_nc():
    global _nc_cache
    if _nc_cache is None:
        _nc_cache = _build()
    return _nc_cache


def kernel(hidden_states, positions, w_q_a, q_a_ln_w, w_q_b, w_kv_a,
           kv_a_ln_w, w_kc, w_vc, w_o):
    global _last_results
    hidden_states = np.asarray(hidden_states, dtype=np.float32)
    positions = np.asarray(positions)
    w_q_a = np.asarray(w_q_a, dtype=np.float32)
    q_a_ln_w = np.asarray(q_a_ln_w, dtype=np.float32)
    w_q_b = np.asarray(w_q_b, dtype=np.float32)
    w_kv_a = np.asarray(w_kv_a, dtype=np.float32)
    kv_a_ln_w = np.asarray(kv_a_ln_w, dtype=np.float32)
    w_kc = np.asarray(w_kc, dtype=np.float32)
    w_vc = np.asarray(w_vc, dtype=np.float32)
    w_o = np.asarray(w_o, dtype=np.float32)

    perm = _pe_perm()
    wqb_h = w_q_b * q_a_ln_w[:, None]
    wqb_h = wqb_h.reshape(QLR, H, QKD)
    wqb_h = np.concatenate([wqb_h[:, :, :NOPE], wqb_h[:, :, NOPE:][:, :, perm]],
                           axis=2)
    wqb_h = round_fp32r(wqb_h.reshape(QLR, H * QKD))
    wkva_h = np.concatenate([w_kv_a[:, :KVLR], w_kv_a[:, KVLR:][:, perm]], axis=1)
    wkva_h = round_fp32r(wkva_h)
    wkc_h = round_fp32r(w_kc * kv_a_ln_w[None, None, :])
    wvc_h = round_fp32r(w_vc * kv_a_ln_w[None, :, None])
    wqa_h = round_fp32r(w_q_a)
    wo_h = round_fp32r(w_o)

    inv_freq = 1.0 / (THETA ** (np.arange(0, ROPE, 2, dtype=np.float64) / ROPE))
    freqs = positions.astype(np.float64)[:, None] * inv_freq[None, :]
    cos = round_fp32r(np.cos(freqs).astype(np.float32))
    sin = round_fp32r(np.sin(freqs).astype(np.float32))

    tri = np.tril(np.ones((128, 128), dtype=np.float32))
    in_maps = []
    for c in range(N_CORES):
        gbs = (c, 15 - c)
        tok = np.concatenate([np.arange(g * 128, (g + 1) * 128) for g in gbs])
        hid_own = round_fp32r(hidden_states[tok])
        cs_own = np.concatenate([cos[tok], sin[tok]], axis=1)
        csrep = np.concatenate([np.tile(cos[tok], (1, H)),
                                np.tile(sin[tok], (1, H))], axis=1)
        bmask = np.zeros((128, KMAX[0] + KMAX[1]), dtype=np.float32)
        for slot, g in enumerate(gbs):
            kmax = (g + 1) * 128
            off = 0 if slot == 0 else KMAX[0]
            bmask[:, off:off + kmax] = 1.0
            bmask[:, off + kmax - 128:off + kmax] = tri
        in_maps.append({
            "hid": hid_own, "wqa": wqa_h, "wqb": wqb_h, "wkva": wkva_h,
            "wkc": wkc_h, "wvc": wvc_h, "wo": wo_h,
            "cs_own": cs_own.astype(np.float32),
            "csrep": csrep.astype(np.float32), "bmask": bmask,
        })

    nc = _get_nc()
    trace = bool(os.environ.get("MLA_TRACE"))
    if trace:
        try:
            import prof_shim
            prof_shim.install()
        except Exception:
            trace = False
    res = run_bass_kernel_spmd(nc, in_maps, list(range(N_CORES)), trace=trace)
    _last_results = res

    outp = np.empty((T, HID), dtype=np.float32)
    for c in range(N_CORES):
        outT = res.results[c]["outT"]
        for pos, g in ((0, c), (1, 15 - c)):
            outp[g * 128:(g + 1) * 128] = outT[:, pos * 128:(pos + 1) * 128].T
    return outp


# revision 4
# speedup vs baseline: 1.1955x; 1.1955x over previous
"""DeepseekV2 MLA prefill kernel for 8 Trainium2 NeuronCores.

Sharding: token-parallel over causal query blocks. Core c owns query blocks
{c, 15-c} (128 tokens each -> balanced causal work: 17 key-blocks per core).
Each core computes the shared latent KV only for its own 256 tokens; the tiny
[576, 256] latent-KV slice is AllGathered (fp32r, ~590KB/rank), then every
core runs all 32 heads over its own query blocks and the full o_proj for its
tokens. The host reassembles disjoint token slices - no all-reduce needed.

SPMD note: all cores run one program, so per-core causal lengths are handled
by padding scores to fixed widths (1024 for the low block, 2048 for the high
block) and zeroing the surplus via per-core uploaded column masks.

All matmuls run in fp32r (fp32 with 11-bit mantissa, full PE rate at N>=256).
Weights are pre-rounded host-side; rmsnorm ln-weights are folded into the
adjacent weight matrices; RoPE pair de-interleave is folded into weight column
permutations; cos/sin tables are computed from `positions` on the host.
"""

import os
import numpy as np

import concourse.bacc as bacc
import concourse.bass as bass
import concourse.mybir as mybir
import concourse.tile as tile
from concourse.bass_utils import run_bass_kernel_spmd
from concourse.masks import make_identity

F32R = mybir.dt.float32r
F32 = mybir.dt.float32
AOP = mybir.AluOpType
AFT = mybir.ActivationFunctionType

HID = 5120
QLR = 1536
KVLR = 512
NOPE = 128
ROPE = 64
VDIM = 128
H = 32
T = 2048
QKD = NOPE + ROPE
SCALE = QKD ** -0.5
EPS = 1e-6
THETA = 10000.0

N_CORES = 8
NB = T // 128          # 16 key/query blocks of 128
KD = HID // 128        # 40
KQ = QLR // 128        # 12
LKV = KVLR + ROPE      # 576
KMAX = (1024, 2048)    # padded score widths for block slots {c, 15-c}

_last_results = None


def round_fp32r(x):
    b = np.ascontiguousarray(x, dtype=np.float32).view(np.uint32)
    lsb = (b >> 12) & 1
    return ((b + np.uint32(0x7FF) + lsb) & np.uint32(0xFFFFF000)).view(np.float32)


def _pe_perm():
    return np.concatenate([np.arange(0, ROPE, 2), np.arange(1, ROPE, 2)])


def _build():
    nc = bacc.Bacc("TRN2", target_bir_lowering=False, debug=False,
                   num_devices=N_CORES)

    hid = nc.dram_tensor("hid", [256, HID], F32R, kind="ExternalInput").ap()
    wqa = nc.dram_tensor("wqa", [HID, QLR], F32R, kind="ExternalInput").ap()
    wqb = nc.dram_tensor("wqb", [QLR, H * QKD], F32R, kind="ExternalInput").ap()
    wkva = nc.dram_tensor("wkva", [HID, LKV], F32R, kind="ExternalInput").ap()
    wkc = nc.dram_tensor("wkc", [H, NOPE, KVLR], F32R, kind="ExternalInput").ap()
    wvc = nc.dram_tensor("wvc", [H, 128, 4 * VDIM], F32R, kind="ExternalInput").ap()
    wo = nc.dram_tensor("wo", [KD, 128, H * 128], F32R, kind="ExternalInput").ap()
    cs_own = nc.dram_tensor("cs_own", [256, ROPE], F32R, kind="ExternalInput").ap()
    csrep = nc.dram_tensor("csrep", [256, 2 * H * 32], F32R,
                           kind="ExternalInput").ap()
    bmask = nc.dram_tensor("bmask", [128, KMAX[0] + KMAX[1]], F32R,
                           kind="ExternalInput").ap()
    out = nc.dram_tensor("outT", [HID, 256], F32, kind="ExternalOutput").ap()

    with tile.TileContext(nc) as tc:
        with tc.tile_pool(name="outer", bufs=1) as pp, \
             tc.tile_pool(name="dram", bufs=1, space="DRAM") as dram:
            # ---- constants + tensors alive through P6 ----
            ident_f = pp.tile([128, 128], F32)
            make_identity(nc, ident_f[:])
            ident = pp.tile([128, 128], F32R)
            nc.vector.tensor_copy(ident[:], ident_f[:])
            eps_t = pp.tile([128, 1], F32)
            nc.gpsimd.memset(eps_t[:], EPS)
            cs_sb = pp.tile([128, 2, ROPE], F32R)
            nc.sync.dma_start(cs_sb[:], cs_own.rearrange("(b p) r -> p b r", p=128))
            attn_all = pp.tile([128, H, 256], F32R)

            in_blob = dram.tile([LKV, 256], F32R)
            gblob = dram.tile([N_CORES * LKV, 256], F32R, addr_space="Shared")

            with tc.tile_pool(name="kvpool", bufs=1) as kv:
                kT = [kv.tile([128, T], F32R, name=f"kT{i}") for i in range(4)]
                kTpe = kv.tile([64, T], F32R)
                qanT = kv.tile([128, KQ, 256], F32R)

                # ======== P1 + P2 (hT-scoped) ========
                with tc.tile_pool(name="hpool", bufs=1) as hp:
                    hT = hp.tile([128, 2, KD, 128], F32R)
                    kTo = [hp.tile([128, 256], F32R, name=f"kTo{i}")
                           for i in range(4)]
                    kToPe = hp.tile([64, 256], F32R)
                    with tc.tile_pool(name="p1ps", bufs=1, space="PSUM") as p1ps:
                        for b in range(2):
                            # transpose own hidden rows -> hT (stream quarters)
                            for q4 in range(4):
                                hsb = hp.tile([128, HID // 4], F32R,
                                              tag="hsb", bufs=2)
                                nc.sync.dma_start(
                                    hsb[:], hid[b * 128:(b + 1) * 128,
                                                q4 * (HID // 4):(q4 + 1) * (HID // 4)])
                                for kk in range(KD // 4):
                                    k = q4 * (KD // 4) + kk
                                    trp = p1ps.tile([128, 128], F32R,
                                                    tag="tr", bufs=2)
                                    nc.tensor.transpose(
                                        trp[:], hsb[:, kk * 128:(kk + 1) * 128],
                                        ident[:])
                                    nc.vector.tensor_copy(hT[:, b, k, :], trp[:])
                            # latent = hid @ wkva (token-major)
                            psA = p1ps.tile([128, 512], F32, tag="acc", bufs=2)
                            psB = p1ps.tile([128, 64], F32, tag="accB", bufs=2)
                            for k in range(KD):
                                wk = hp.tile([128, LKV], F32R, tag="wkva", bufs=4)
                                nc.sync.dma_start(
                                    wk[:], wkva[k * 128:(k + 1) * 128, :])
                                nc.tensor.matmul(psA[:], hT[:, b, k, :],
                                                 wk[:, 0:KVLR], start=(k == 0),
                                                 stop=(k == KD - 1),
                                                 skip_group_check=True)
                                nc.tensor.matmul(psB[:], hT[:, b, k, :],
                                                 wk[:, KVLR:LKV], start=(k == 0),
                                                 stop=(k == KD - 1),
                                                 skip_group_check=True)
                            # rmsnorm(ckv)
                            ckv = hp.tile([128, KVLR], F32, tag="ckv", bufs=2)
                            nc.vector.tensor_copy(ckv[:], psA[:])
                            sq = hp.tile([128, QLR], F32, tag="sqscr", bufs=1)
                            ssq = hp.tile([128, 1], F32, tag="ssq", bufs=2)
                            nc.scalar.activation(sq[:, 0:KVLR], ckv[:], AFT.Square,
                                                 accum_out=ssq[:])
                            den = hp.tile([128, 1], F32, tag="den", bufs=2)
                            nc.scalar.activation(den[:], ssq[:], AFT.Sqrt,
                                                 scale=1.0 / KVLR, bias=eps_t[:])
                            fac = hp.tile([128, 1], F32, tag="fac", bufs=2)
                            nc.vector.reciprocal(fac[:], den[:])
                            vown = hp.tile([128, KVLR], F32R, tag="vown", bufs=2)
                            nc.vector.tensor_scalar_mul(vown[:], ckv[:], fac[:])
                            # rope(k_pe) - psB holds deinterleaved [x1|x2]
                            kpe = hp.tile([128, ROPE], F32R, tag="kpe", bufs=2)
                            t1 = hp.tile([128, 32], F32, tag="t1", bufs=2)
                            t2 = hp.tile([128, 32], F32, tag="t2", bufs=2)
                            cosv = cs_sb[:, b, 0:32]
                            sinv = cs_sb[:, b, 32:64]
                            nc.vector.tensor_tensor(out=t1[:], in0=psB[:, 0:32],
                                                    in1=cosv, op=AOP.mult)
                            nc.vector.tensor_tensor(out=t2[:], in0=psB[:, 32:64],
                                                    in1=sinv, op=AOP.mult)
                            nc.vector.tensor_tensor(out=kpe[:, 0:32], in0=t1[:],
                                                    in1=t2[:], op=AOP.subtract)
                            nc.vector.tensor_tensor(out=t1[:], in0=psB[:, 32:64],
                                                    in1=cosv, op=AOP.mult)
                            nc.vector.tensor_tensor(out=t2[:], in0=psB[:, 0:32],
                                                    in1=sinv, op=AOP.mult)
                            nc.vector.tensor_tensor(out=kpe[:, 32:64], in0=t1[:],
                                                    in1=t2[:], op=AOP.add)
                            # transpose vown + kpe -> own latent-KV columns
                            for kc in range(4):
                                trp = p1ps.tile([128, 128], F32R, tag="tr", bufs=2)
                                nc.tensor.transpose(
                                    trp[:], vown[:, kc * 128:(kc + 1) * 128],
                                    ident[:])
                                nc.vector.tensor_copy(
                                    kTo[kc][:, b * 128:(b + 1) * 128], trp[:])
                            trp2 = p1ps.tile([64, 128], F32R, tag="tr", bufs=2)
                            nc.tensor.transpose(trp2[:], kpe[:], ident[:])
                            nc.vector.tensor_copy(
                                kToPe[:, b * 128:(b + 1) * 128], trp2[:])

                        # gather latent KV across cores
                        for kc in range(4):
                            nc.gpsimd.dma_start(
                                in_blob[kc * 128:(kc + 1) * 128, :], kTo[kc][:])
                        nc.gpsimd.dma_start(in_blob[KVLR:LKV, :], kToPe[:])
                        nc.gpsimd.collective_compute(
                            "AllGather", AOP.bypass,
                            replica_groups=[list(range(N_CORES))],
                            ins=[in_blob.opt()], outs=[gblob.opt()],
                        )
                        for r in range(N_CORES):
                            for pos, gb in ((0, r), (1, 15 - r)):
                                src = gblob[r * LKV:(r + 1) * LKV,
                                            pos * 128:(pos + 1) * 128]
                                for kc in range(4):
                                    nc.sync.dma_start(
                                        kT[kc][:, gb * 128:(gb + 1) * 128],
                                        src[kc * 128:(kc + 1) * 128, :])
                                nc.sync.dma_start(
                                    kTpe[:, gb * 128:(gb + 1) * 128],
                                    src[KVLR:LKV, :])

                    # ---- P2: q_a for own tokens (uses hT) ----
                    with tc.tile_pool(name="p2ps", bufs=1, space="PSUM") as p2ps:
                        qaps = [[p2ps.tile([128, 512], F32, tag=f"qa{b}{j}",
                                           name=f"qaps{b}{j}")
                                 for j in range(3)] for b in range(2)]
                        for k in range(KD):
                            wq = hp.tile([128, QLR], F32R, tag="wqa", bufs=3)
                            nc.sync.dma_start(wq[:], wqa[k * 128:(k + 1) * 128, :])
                            for b in range(2):
                                for j in range(3):
                                    nc.tensor.matmul(
                                        qaps[b][j][:], hT[:, b, k, :],
                                        wq[:, j * 512:(j + 1) * 512],
                                        start=(k == 0), stop=(k == KD - 1),
                                        skip_group_check=True)
                        for b in range(2):
                            qa_sb = hp.tile([128, QLR], F32, tag="qasb", bufs=1)
                            for j in range(3):
                                nc.vector.tensor_copy(
                                    qa_sb[:, j * 512:(j + 1) * 512], qaps[b][j][:])
                            sq2 = hp.tile([128, QLR], F32, tag="sqscr", bufs=1)
                            ss2 = hp.tile([128, 1], F32, tag="ss2", bufs=2)
                            nc.scalar.activation(sq2[:], qa_sb[:], AFT.Square,
                                                 accum_out=ss2[:])
                            den2 = hp.tile([128, 1], F32, tag="den2", bufs=2)
                            nc.scalar.activation(den2[:], ss2[:], AFT.Sqrt,
                                                 scale=1.0 / QLR, bias=eps_t[:])
                            fac2 = hp.tile([128, 1], F32, tag="fac2", bufs=2)
                            nc.vector.reciprocal(fac2[:], den2[:])
                            qan = hp.tile([128, QLR], F32R, tag="qan", bufs=1)
                            nc.vector.tensor_scalar_mul(qan[:], qa_sb[:], fac2[:])
                            for j12 in range(KQ):
                                trp = p2ps.tile([128, 128], F32R, tag="tr", bufs=2)
                                nc.tensor.transpose(
                                    trp[:], qan[:, j12 * 128:(j12 + 1) * 128],
                                    ident[:])
                                nc.vector.tensor_copy(
                                    qanT[:, j12, b * 128:(b + 1) * 128], trp[:])

                # ======== P3: q = q_a_n @ wqb (token-major), hpool freed ======
                with tc.tile_pool(name="vpool", bufs=1) as vp:
                    q_all = vp.tile([128, 2, H * QKD], F32R)
                    vall = vp.tile([128, NB, KVLR], F32R)
                    with tc.tile_pool(name="p3", bufs=1) as p3, \
                         tc.tile_pool(name="p3ps", bufs=1, space="PSUM") as p3ps:
                        for n in range(KQ):
                            ps0 = p3ps.tile([128, 512], F32, tag="qb0", bufs=2)
                            ps1 = p3ps.tile([128, 512], F32, tag="qb1", bufs=2)
                            for k in range(KQ):
                                wb = p3.tile([128, 512], F32R, tag="wqb", bufs=8)
                                nc.sync.dma_start(
                                    wb[:], wqb[k * 128:(k + 1) * 128,
                                               n * 512:(n + 1) * 512])
                                nc.tensor.matmul(ps0[:], qanT[:, k, 0:128], wb[:],
                                                 start=(k == 0),
                                                 stop=(k == KQ - 1),
                                                 skip_group_check=True)
                                nc.tensor.matmul(ps1[:], qanT[:, k, 128:256],
                                                 wb[:], start=(k == 0),
                                                 stop=(k == KQ - 1),
                                                 skip_group_check=True)
                            nc.vector.tensor_copy(
                                q_all[:, 0, n * 512:(n + 1) * 512], ps0[:])
                            nc.vector.tensor_copy(
                                q_all[:, 1, n * 512:(n + 1) * 512], ps1[:])

                    # ---- P2b: derive token-major v from gathered kT ----
                    with tc.tile_pool(name="p2bps", bufs=1, space="PSUM") as p2b:
                        for sb in range(NB):
                            for kc in range(4):
                                trp = p2b.tile([128, 128], F32R, tag="tr", bufs=4)
                                nc.tensor.transpose(
                                    trp[:], kT[kc][:, sb * 128:(sb + 1) * 128],
                                    ident[:])
                                nc.vector.tensor_copy(
                                    vall[:, sb, kc * 128:(kc + 1) * 128], trp[:])

                    # ---- P4: rope on q_pe across all heads ----
                    with tc.tile_pool(name="p4", bufs=1) as p4:
                        crep = p4.tile([128, 2, H * 32], F32R, tag="crep")
                        srep = p4.tile([128, 2, H * 32], F32R, tag="srep")
                        csr = csrep.rearrange("(b p) (c x) -> p b c x", p=128, c=2)
                        nc.sync.dma_start(crep[:], csr[:, :, 0, :])
                        nc.sync.dma_start(srep[:], csr[:, :, 1, :])
                        for b in range(2):
                            base = q_all[:, b]
                            x1 = bass.AP(base.tensor, base.offset + NOPE,
                                         [base.ap[0], [QKD, H], [1, 32]])
                            x2 = bass.AP(base.tensor, base.offset + NOPE + 32,
                                         [base.ap[0], [QKD, H], [1, 32]])
                            cosv = crep[:, b].rearrange("p (h x) -> p h x", h=H)
                            sinv = srep[:, b].rearrange("p (h x) -> p h x", h=H)
                            t1 = p4.tile([128, H, 32], F32, tag="t1")
                            t2 = p4.tile([128, H, 32], F32, tag="t2")
                            t3 = p4.tile([128, H, 32], F32, tag="t3")
                            t4 = p4.tile([128, H, 32], F32, tag="t4")
                            nc.vector.tensor_tensor(out=t1[:], in0=x1, in1=cosv,
                                                    op=AOP.mult)
                            nc.vector.tensor_tensor(out=t2[:], in0=x2, in1=sinv,
                                                    op=AOP.mult)
                            nc.vector.tensor_tensor(out=t3[:], in0=x2, in1=cosv,
                                                    op=AOP.mult)
                            nc.vector.tensor_tensor(out=t4[:], in0=x1, in1=sinv,
                                                    op=AOP.mult)
                            nc.vector.tensor_tensor(out=x1, in0=t1[:], in1=t2[:],
                                                    op=AOP.subtract)
                            nc.vector.tensor_tensor(out=x2, in0=t3[:], in1=t4[:],
                                                    op=AOP.add)

                    # ---- P5: attention, head-by-head ----
                    with tc.tile_pool(name="p5", bufs=1) as p5, \
                         tc.tile_pool(name="p5ps", bufs=1, space="PSUM") as p5ps:
                        bm = p5.tile([128, KMAX[0] + KMAX[1]], F32R, tag="bm")
                        nc.sync.dma_start(bm[:], bmask[:])
                        bmx = (bm[:, 0:KMAX[0]], bm[:, KMAX[0]:])
                        for h in range(H):
                            wkc_s = p5.tile([128, KVLR], F32R, tag="wkc", bufs=2)
                            nc.sync.dma_start(wkc_s[:], wkc[h])
                            wvc_s = p5.tile([128, 4, VDIM], F32R, tag="wvc", bufs=2)
                            nc.sync.dma_start(wvc_s[:], wvc[h])
                            qnT = p5.tile([128, 256], F32R, tag="qnT", bufs=1)
                            qpeT = p5.tile([64, 256], F32R, tag="qpeT", bufs=1)
                            for b in range(2):
                                trp = p5ps.tile([128, 128], F32R, tag="tr", bufs=2)
                                nc.tensor.transpose(
                                    trp[:], q_all[:, b, h * QKD:h * QKD + NOPE],
                                    ident[:])
                                nc.vector.tensor_copy(
                                    qnT[:, b * 128:(b + 1) * 128], trp[:])
                                trp2 = p5ps.tile([64, 128], F32R, tag="tr", bufs=2)
                                nc.tensor.transpose(
                                    trp2[:],
                                    q_all[:, b, h * QKD + NOPE:(h + 1) * QKD],
                                    ident[:])
                                nc.vector.tensor_copy(
                                    qpeT[:, b * 128:(b + 1) * 128], trp2[:])
                            qloT = p5.tile([128, 4, 256], F32R, tag="qloT", bufs=1)
                            for m in range(4):
                                aps = p5ps.tile([128, 256], F32, tag="acc", bufs=2)
                                nc.tensor.matmul(aps[:],
                                                 wkc_s[:, m * 128:(m + 1) * 128],
                                                 qnT[:], start=True, stop=True)
                                nc.vector.tensor_copy(qloT[:, m], aps[:])

                            ctxT = p5.tile([128, 4, 256], F32R, tag="ctxT", bufs=1)
                            for b in range(2):
                                kmax = KMAX[b]
                                nst = kmax // 512
                                probs = p5.tile([128, 2048], F32R, tag="probs",
                                                bufs=1)
                                partials = p5.tile([128, 4], F32, tag="part",
                                                   bufs=2)
                                for st in range(nst):
                                    sl = slice(st * 512, (st + 1) * 512)
                                    sc_ps = p5ps.tile([128, 512], F32, tag="sc",
                                                      bufs=4)
                                    for kc in range(5):
                                        if kc < 4:
                                            lhs = qloT[:, kc][:, b * 128:(b + 1) * 128]
                                            rhs = kT[kc][:, sl]
                                        else:
                                            lhs = qpeT[:, b * 128:(b + 1) * 128]
                                            rhs = kTpe[:, sl]
                                        nc.tensor.matmul(
                                            sc_ps[:], lhs, rhs, start=(kc == 0),
                                            stop=(kc == 4))
                                    nc.scalar.activation(probs[:, sl], sc_ps[:],
                                                         AFT.Exp, scale=SCALE)
                                    nc.vector.tensor_tensor(
                                        out=probs[:, sl], in0=probs[:, sl],
                                        in1=bmx[b][:, sl], op=AOP.mult)
                                    nc.vector.tensor_reduce(
                                        out=partials[:, st:st + 1],
                                        in_=probs[:, sl],
                                        axis=mybir.AxisListType.X, op=AOP.add)
                                sumexp = p5.tile([128, 1], F32, tag="sume", bufs=2)
                                nc.vector.tensor_reduce(
                                    out=sumexp[:], in_=partials[:, 0:nst],
                                    axis=mybir.AxisListType.X, op=AOP.add)
                                recip = p5.tile([128, 1], F32, tag="rec", bufs=2)
                                nc.vector.reciprocal(recip[:], sumexp[:])
                                ctx_ps = p5ps.tile([128, 512], F32, tag="acc",
                                                   bufs=2)
                                for sc in range(kmax // 128):
                                    trp = p5ps.tile([128, 128], F32R, tag="tr",
                                                    bufs=2)
                                    nc.tensor.transpose(
                                        trp[:], probs[:, sc * 128:(sc + 1) * 128],
                                        ident[:])
                                    pT = p5.tile([128, 128], F32R, tag="pT",
                                                 bufs=2)
                                    nc.vector.tensor_copy(pT[:], trp[:])
                                    nc.tensor.matmul(
                                        ctx_ps[:], pT[:], vall[:, sc],
                                        start=(sc == 0),
                                        stop=(sc == kmax // 128 - 1))
                                ctx_sb = p5.tile([128, KVLR], F32R, tag="ctxsb",
                                                 bufs=1)
                                nc.vector.tensor_scalar_mul(ctx_sb[:], ctx_ps[:],
                                                            recip[:])
                                for rc in range(4):
                                    trp = p5ps.tile([128, 128], F32R, tag="tr",
                                                    bufs=2)
                                    nc.tensor.transpose(
                                        trp[:], ctx_sb[:, rc * 128:(rc + 1) * 128],
                                        ident[:])
                                    nc.vector.tensor_copy(
                                        ctxT[:, rc, b * 128:(b + 1) * 128], trp[:])
                            atps = p5ps.tile([128, 256], F32, tag="acc", bufs=2)
                            for rc in range(4):
                                nc.tensor.matmul(atps[:], wvc_s[:, rc],
                                                 ctxT[:, rc], start=(rc == 0),
                                                 stop=(rc == 3))
                            nc.vector.tensor_copy(attn_all[:, h], atps[:])

            # ======== P6: o_proj (kvpool freed -> room for wo stream) ========
            with tc.tile_pool(name="p6", bufs=1) as p6, \
                 tc.tile_pool(name="p6ps", bufs=1, space="PSUM") as p6ps:
                for m in range(KD):
                    wom = p6.tile([128, H, 128], F32R, tag="wo", bufs=3)
                    nc.sync.dma_start(wom[:], wo[m])
                    ops_ = p6ps.tile([128, 256], F32, tag="acc", bufs=2)
                    for h in range(H):
                        nc.tensor.matmul(ops_[:], wom[:, h], attn_all[:, h],
                                         start=(h == 0), stop=(h == H - 1))
                    osb = p6.tile([128, 256], F32, tag="osb", bufs=3)
                    nc.vector.tensor_copy(osb[:], ops_[:])
                    nc.sync.dma_start(out[m * 128:(m + 1) * 128, :], osb[:])

    nc.compile()
    return nc


_nc_cache = None


def _get_nc():
    global _nc_cache
    if _nc_cache is None:
        _nc_cache = _build()
    return _nc_cache


def kernel(hidden_states, positions, w_q_a, q_a_ln_w, w_q_b, w_kv_a,
           kv_a_ln_w, w_kc, w_vc, w_o):
    global _last_results
    hidden_states = np.asarray(hidden_states, dtype=np.float32)
    positions = np.asarray(positions)
    w_q_a = np.asarray(w_q_a, dtype=np.float32)
    q_a_ln_w = np.asarray(q_a_ln_w, dtype=np.float32)
    w_q_b = np.asarray(w_q_b, dtype=np.float32)
    w_kv_a = np.asarray(w_kv_a, dtype=np.float32)
    kv_a_ln_w = np.asarray(kv_a_ln_w, dtype=np.float32)
    w_kc = np.asarray(w_kc, dtype=np.float32)
    w_vc = np.asarray(w_vc, dtype=np.float32)
    w_o = np.asarray(w_o, dtype=np.float32)

    perm = _pe_perm()
    wqb_h = w_q_b * q_a_ln_w[:, None]
    wqb_h = wqb_h.reshape(QLR, H, QKD)
    wqb_h = np.concatenate([wqb_h[:, :, :NOPE], wqb_h[:, :, NOPE:][:, :, perm]],
                           axis=2)
    wqb_h = round_fp32r(wqb_h.reshape(QLR, H * QKD))
    wkva_h = np.concatenate([w_kv_a[:, :KVLR], w_kv_a[:, KVLR:][:, perm]], axis=1)
    wkva_h = round_fp32r(wkva_h)
    wkc_h = round_fp32r(w_kc * kv_a_ln_w[None, None, :])
    # wvc packed: [h, s, c*VDIM+v] = wvc'[h, c*128+s, v]
    wvc_h = round_fp32r(
        (w_vc * kv_a_ln_w[None, :, None])
        .reshape(H, 4, 128, VDIM).transpose(0, 2, 1, 3).reshape(H, 128, 4 * VDIM))
    wqa_h = round_fp32r(w_q_a)
    # wo packed: [m, s, h*128+v] = w_o[h*128+s, m*128+v]
    wo_h = round_fp32r(
        w_o.reshape(H, 128, KD, 128).transpose(2, 1, 0, 3).reshape(KD, 128, H * 128))

    inv_freq = 1.0 / (THETA ** (np.arange(0, ROPE, 2, dtype=np.float64) / ROPE))
    freqs = positions.astype(np.float64)[:, None] * inv_freq[None, :]
    cos = round_fp32r(np.cos(freqs).astype(np.float32))
    sin = round_fp32r(np.sin(freqs).astype(np.float32))

    tri = np.tril(np.ones((128, 128), dtype=np.float32))
    in_maps = []
    for c in range(N_CORES):
        gbs = (c, 15 - c)
        tok = np.concatenate([np.arange(g * 128, (g + 1) * 128) for g in gbs])
        hid_own = round_fp32r(hidden_states[tok])
        cs_own = np.concatenate([cos[tok], sin[tok]], axis=1)
        csrep = np.concatenate([np.tile(cos[tok], (1, H)),
                                np.tile(sin[tok], (1, H))], axis=1)
        bmask = np.zeros((128, KMAX[0] + KMAX[1]), dtype=np.float32)
        for slot, g in enumerate(gbs):
            kmax = (g + 1) * 128
            off = 0 if slot == 0 else KMAX[0]
            bmask[:, off:off + kmax] = 1.0
            bmask[:, off + kmax - 128:off + kmax] = tri
        in_maps.append({
            "hid": hid_own, "wqa": wqa_h, "wqb": wqb_h, "wkva": wkva_h,
            "wkc": wkc_h, "wvc": wvc_h, "wo": wo_h,
            "cs_own": cs_own.astype(np.float32),
            "csrep": csrep.astype(np.float32), "bmask": bmask,
        })

    nc = _get_nc()
    trace = bool(os.environ.get("MLA_TRACE"))
    if trace:
        try:
            import prof_shim
            prof_shim.install()
        except Exception:
            trace = False
    res = run_bass_kernel_spmd(nc, in_maps, list(range(N_CORES)), trace=trace)
    _last_results = res

    outp = np.empty((T, HID), dtype=np.float32)
    for c in range(N_CORES):
        outT = res.results[c]["outT"]
        for pos, g in ((0, c), (1, 15 - c)):
            outp[g * 128:(g + 1) * 128] = outT[:, pos * 128:(pos + 1) * 128].T
    return outp
